# revision 25
# baseline (speedup 1.0000x reference)
"""CGCNN (3x CGConv + graph LayerNorm + global mean pool + MLP) on 8 TRN2 NeuronCores.

Strategy (graph/data parallel, per sharding hint):
  - Nodes partitioned contiguously across 8 cores (6250 each, padded to 6272 = 49*128).
  - Edges sharded by destination node; per (dst-block, src-table-half) grouping,
    sorted+padded host-side into a uniform cross-core static schedule.
  - CGConv z@W is split: per-node projections (dst part + src part) computed as
    node-level matmuls; src projections AllGathered into a global bf16 table;
    per-edge rows fetched with batched dma_gather (int16 idx -> table split in 2
    halves of 25088 rows); edge_attr part is a per-tile PE matmul.
  - segment_sum by dst via one-hot matmul into PSUM per 128-node block.
  - Graph LayerNorm: local sum/sumsq partials + AllReduce of 2 scalars.
  - Readout: one-hot-by-graph-id matmul partial pooled sums + AllReduce; small MLP.

Numerics: edge pipeline in bf16 (fp32 PSUM accumulation), node pipeline fp32.
"""

import numpy as np
import ml_dtypes

import concourse.bass as bass
import concourse.bacc as bacc
import concourse.mybir as mybir
import concourse.tile as tile
from concourse.bass_utils import run_bass_kernel_spmd
from concourse.library_config import mlp as _mlp_lib

BF16 = ml_dtypes.bfloat16
NC_CORES = 8
BLK = 128
GCHUNK = 1024          # rows per dma_gather call (Q7 scratch limit < 2048)
ECHUNK_TILES = 32      # eT tiles per DMA chunk
EPS = 1e-5

F32 = mybir.dt.float32
BF = mybir.dt.bfloat16
I16 = mybir.dt.int16
AF = mybir.ActivationFunctionType
OP = mybir.AluOpType


def _ceil_div(a, b):
    return (a + b - 1) // b


def _wrap_idx16(idx, nchunks):
    """Pack int16 indices into the dma_gather SBUF layout.

    Stream of nchunks*GCHUNK indices -> [128, nchunks*(GCHUNK//16)] int16:
    per chunk, wrap 16 partitions and replicate 8x down the partition dim.
    """
    cols = GCHUNK // 16
    out = np.zeros((128, nchunks * cols), np.int16)
    for c in range(nchunks):
        w = idx[c * GCHUNK : (c + 1) * GCHUNK].reshape(cols, 16).T  # [16, cols]
        out[:, c * cols : (c + 1) * cols] = np.tile(w, (8, 1))
    return out


def host_prep(x, edge_index, edge_attr, batch, W1, b1, layer_params, W2, b2, W3, b3):
    """Build the uniform cross-core schedule and all per-core input arrays."""
    N, Din = x.shape
    E = edge_index.shape[1]
    G = 128
    D = 128
    De = edge_attr.shape[1]
    NP = N // NC_CORES
    assert NP * NC_CORES == N
    NPP = _ceil_div(NP, BLK) * BLK
    NBLK = NPP // BLK
    HALF = 4 * NPP  # rows per gather-table half
    assert HALF <= 32767

    src = np.asarray(edge_index[0], np.int64)
    dst = np.asarray(edge_index[1], np.int64)
    batch = np.asarray(batch, np.int64)
    x = np.asarray(x, np.float32)
    edge_attr = np.asarray(edge_attr, np.float32)

    # padded-global src index (into the AllGather'd table)
    src_pad = (src // NP) * NPP + (src % NP)
    half_of_src = (src_pad >= HALF).astype(np.int64)

    # per-core edge partition (by dst owner), grouped by (block, half)
    per_core = []
    counts = np.zeros((NC_CORES, NBLK, 2), np.int64)
    for c in range(NC_CORES):
        m = (dst // NP) == c
        e_ids = np.nonzero(m)[0]
        dl = dst[e_ids] - c * NP
        b = dl // BLK
        h = half_of_src[e_ids]
        order = np.lexsort((dl, h, b))
        e_ids = e_ids[order]
        dl = dl[order]
        b = b[order]
        h = h[order]
        per_core.append((e_ids, dl, b, h))
        for bb in range(NBLK):
            mb = b == bb
            counts[c, bb, 0] = np.count_nonzero(mb & (h == 0))
            counts[c, bb, 1] = np.count_nonzero(mb & (h == 1))

    ntiles = _ceil_div(np.max(counts, axis=0), BLK)  # [NBLK, 2]
    nA = ntiles[:, 0].astype(int)
    nB = ntiles[:, 1].astype(int)
    TOT = int(nA.sum() + nB.sum())
    SLOTS = TOT * BLK
    TA = int(nA.sum())
    TB = int(nB.sum())
    NCA = _ceil_div(TA * BLK, GCHUNK)
    NCB = _ceil_div(TB * BLK, GCHUNK)
    NCD = _ceil_div(SLOTS, GCHUNK)
    NCE = _ceil_div(TOT, ECHUNK_TILES)

    sched = dict(
        N=N, E=E, G=G, D=D, De=De, Din=Din, NP=NP, NPP=NPP, NBLK=NBLK, HALF=HALF,
        nA=nA, nB=nB, TOT=TOT, TA=TA, TB=TB,
        NCA=NCA, NCB=NCB, NCD=NCD, NCE=NCE,
    )

    cnts = np.bincount(batch, minlength=G).astype(np.float32)
    inv_cnt = (1.0 / np.maximum(cnts, 1.0)).astype(np.float32)

    in_maps = []
    for c in range(NC_CORES):
        e_ids, dl, b, h = per_core[c]
        # slot arrays
        sA = np.zeros(TA * BLK, np.int16)          # idx into table half A
        sB = np.zeros(TB * BLK, np.int16)          # idx into table half B
        sD = np.zeros(SLOTS, np.int16)             # idx into local dst table
        dloc = np.full(SLOTS, -1.0, np.float32)    # dst offset within block
        eperm = np.zeros(SLOTS, np.int64) - 1      # edge id per slot (-1 = pad)
        pA = 0
        pB = 0
        t = 0
        ptr = 0
        for bb in range(NBLK):
            for half, (ntile, sidx, pbase) in enumerate(
                ((nA[bb], sA, pA), (nB[bb], sB, pB))
            ):
                mb_end = ptr
                while mb_end < len(b) and b[mb_end] == bb and h[mb_end] == half:
                    mb_end += 1
                cnt = mb_end - ptr
                assert cnt <= ntile * BLK
                slot0 = t * BLK
                gpos0 = pbase * BLK
                if cnt:
                    ids = e_ids[ptr:mb_end]
                    sidx[gpos0 : gpos0 + cnt] = (
                        src_pad[ids] - half * HALF
                    ).astype(np.int16)
                    dloc[slot0 : slot0 + cnt] = (dl[ptr:mb_end] % BLK).astype(np.float32)
                    sD[slot0 : slot0 + cnt] = dl[ptr:mb_end].astype(np.int16)
                    eperm[slot0 : slot0 + cnt] = ids
                ptr = mb_end
                t += ntile
                if half == 0:
                    pA += ntile
                else:
                    pB += ntile
        assert ptr == len(b)
        assert t == TOT and pA == TA and pB == TB

        # scatter per-half slot arrays into stream layouts
        # stream A slots = concat of A tiles in tile order == sA as built (gpos)
        idxA = _wrap_idx16(np.pad(sA, (0, NCA * GCHUNK - len(sA))), NCA)
        idxB = (
            _wrap_idx16(np.pad(sB, (0, NCB * GCHUNK - len(sB))), NCB)
            if NCB else np.zeros((128, 64), np.int16)
        )
        idxD = _wrap_idx16(np.pad(sD, (0, NCD * GCHUNK - len(sD))), NCD)

        # eT stream [De, SLOTS]
        eT = np.zeros((De, SLOTS), np.float32)
        real = eperm >= 0
        eT[:, real] = edge_attr[eperm[real]].T
        # dloc columns [128, TOT]
        dloc_cols = dloc.reshape(TOT, BLK).T.copy()

        # node-level inputs
        xT = np.zeros((Din + 1, NPP), np.float32)
        xT[:Din, :NP] = x[c * NP : (c + 1) * NP].T
        xT[Din, :NP] = 1.0
        W1a = np.concatenate([W1, b1[None, :]], axis=0).astype(np.float32)

        bc = np.full(NPP, -1.0, np.float32)
        bc[:NP] = batch[c * NP : (c + 1) * NP].astype(np.float32)
        bcols = bc.reshape(NBLK, BLK).T.copy()

        m = {
            "xT": xT,
            "W1a": W1a,
            "idxA": idxA.astype(np.int16),
            "idxB": idxB.astype(np.int16),
            "idxD": idxD.astype(np.int16),
            "dloc": dloc_cols.astype(BF16),
            "eT": eT.astype(BF16),
            "iota": np.tile(np.arange(128, dtype=np.float32)[None, :], (128, 1)).astype(BF16),
            "ident": np.eye(128, dtype=np.float32),
            "ones_col": np.ones((128, 1), np.float32),
            "ones_row": np.ones((1, 128), np.float32),
            "bcols": bcols.astype(np.float32),
            "invc": inv_cnt[:, None].astype(np.float32),
            "W2": W2.astype(np.float32),
            "b2b": np.tile(b2[None, :], (128, 1)).astype(np.float32),
            "W3": W3.astype(np.float32),
            "b3c": np.tile(b3[None, :], (128, 1)).astype(np.float32),
        }
        for li, (Wf, bf, Ws, bs, gw, gb) in enumerate(layer_params, start=1):
            # F-halves negated: the edge-phase exp computes e^(-F) | e^(S)
            m[f"wdst{li}"] = np.concatenate([-Wf[:D], Ws[:D]], axis=1).astype(np.float32)
            m[f"wsrc{li}"] = np.concatenate([-Wf[D : 2 * D], Ws[D : 2 * D]], axis=1).astype(np.float32)
            m[f"bdst{li}"] = np.tile(
                np.concatenate([-bf, bs])[None, :], (128, 1)
            ).astype(np.float32)
            m[f"wefs{li}"] = np.concatenate([-Wf[2 * D :], Ws[2 * D :]], axis=1).astype(BF16)
            m[f"gw{li}"] = gw[:, None].astype(np.float32)
            m[f"gb{li}"] = gb[:, None].astype(np.float32)
        in_maps.append(m)

    return sched, in_maps


def build_nc(sched, n_layers=3, repeat=1):
    N = sched["N"]
    D = sched["D"]
    De = sched["De"]
    G = sched["G"]
    NPP = sched["NPP"]
    NBLK = sched["NBLK"]
    HALF = sched["HALF"]
    nA, nB = sched["nA"], sched["nB"]
    TOT, TA, TB = sched["TOT"], sched["TA"], sched["TB"]
    NCA, NCB, NCD, NCE = sched["NCA"], sched["NCB"], sched["NCD"], sched["NCE"]
    SLOTS = TOT * BLK
    Din1 = sched["Din"] + 1

    import time as _time
    _t0 = _time.time()
    nc = bacc.Bacc("TRN2", target_bir_lowering=False, debug=False,
                   num_devices=NC_CORES)
    rg = [list(range(NC_CORES))]

    # ---- external I/O ----
    ins = {}

    def inp(name, shape, dt):
        ins[name] = nc.dram_tensor(name, list(shape), dt, kind="ExternalInput")
        return ins[name]

    xT_d = inp("xT", (Din1, NPP), F32)
    W1a_d = inp("W1a", (Din1, D), F32)
    idxA_d = inp("idxA", (128, max(NCA, 1) * (GCHUNK // 16)), I16)
    idxB_d = inp("idxB", (128, max(NCB, 1) * (GCHUNK // 16)), I16)
    idxD_d = inp("idxD", (128, NCD * (GCHUNK // 16)), I16)
    dloc_d = inp("dloc", (128, TOT), BF)
    eT_d = inp("eT", (De, SLOTS), BF)
    iota_d = inp("iota", (128, 128), BF)
    ident_d = inp("ident", (128, 128), F32)
    onesc_d = inp("ones_col", (128, 1), F32)
    onesr_d = inp("ones_row", (1, 128), F32)
    bcols_d = inp("bcols", (128, NBLK), F32)
    invc_d = inp("invc", (128, 1), F32)
    W2_d = inp("W2", (D, 16), F32)
    b2b_d = inp("b2b", (128, 16), F32)
    W3_d = inp("W3", (16, 1), F32)
    b3c_d = inp("b3c", (128, 1), F32)
    for li in range(1, n_layers + 1):
        inp(f"wdst{li}", (D, 2 * D), F32)
        inp(f"wsrc{li}", (D, 2 * D), F32)
        inp(f"bdst{li}", (128, 2 * D), F32)
        inp(f"wefs{li}", (De, 2 * D), BF)
        inp(f"gw{li}", (128, 1), F32)
        inp(f"gb{li}", (128, 1), F32)

    out_d = nc.dram_tensor("out", [G, 1], F32, kind="ExternalOutput")

    # ---- internal DRAM ----
    ag_in = [nc.dram_tensor(f"ag_in{li}", [NPP, 2 * D], BF, kind="Internal")
             for li in range(1, n_layers + 1)]
    table = [nc.dram_tensor(f"table{li}", [NC_CORES * NPP, 2 * D], BF,
                            kind="Internal", addr_space="Shared")
             for li in range(1, n_layers + 1)]
    dtab = [nc.dram_tensor(f"dtab{li}", [NPP, 2 * D], BF, kind="Internal")
            for li in range(1, n_layers + 1)]
    st_in = [nc.dram_tensor(f"st_in{li}", [1, 2], F32, kind="Internal")
             for li in range(1, n_layers + 1)]
    st_out = [nc.dram_tensor(f"st_out{li}", [1, 2], F32, kind="Internal",
                             addr_space="Shared")
              for li in range(1, n_layers + 1)]
    pool_in = nc.dram_tensor("pool_in", [G, D], F32, kind="Internal")
    pool_out = nc.dram_tensor("pool_out", [G, D], F32, kind="Internal",
                              addr_space="Shared")

    inv_ND = 1.0 / (float(N) * float(D))

    with tile.TileContext(nc) as tc:
        with (
            tc.tile_pool(name="const", bufs=1) as cpool,
            tc.tile_pool(name="state", bufs=1) as spool,
            tc.tile_pool(name="gath", bufs=3) as gpool,
            tc.tile_pool(name="et", bufs=2) as epool,
            tc.tile_pool(name="ring", bufs=3) as rpool,
            tc.tile_pool(name="work", bufs=2) as wpool,
            tc.tile_pool(name="psB", bufs=3, space="PSUM") as psB,
            tc.tile_pool(name="psC", bufs=2, space="PSUM") as psC,
        ):
            nc.gpsimd.load_library(_mlp_lib)

            def load_const(d, shape, dt, tag):
                t = cpool.tile(list(shape), dt, tag=tag)
                nc.sync.dma_start(t[:], d[:])
                return t

            iota = load_const(iota_d, (128, 128), BF, "iota")
            ident = load_const(ident_d, (128, 128), F32, "ident")
            identb = cpool.tile([128, 128], BF, tag="identb")
            nc.vector.tensor_copy(out=identb[:], in_=ident[:])
            ones_col = load_const(onesc_d, (128, 1), F32, "onesc")
            ones_row = load_const(onesr_d, (1, 128), F32, "onesr")
            dloc = load_const(dloc_d, (128, TOT), BF, "dloc")
            idxA = load_const(idxA_d, idxA_d.shape, I16, "idxA")
            idxB = load_const(idxB_d, idxB_d.shape, I16, "idxB")
            idxD = load_const(idxD_d, idxD_d.shape, I16, "idxD")
            bcols = load_const(bcols_d, (128, NBLK), F32, "bcols")
            invc = load_const(invc_d, (128, 1), F32, "invc")
            W2s = load_const(W2_d, (D, 16), F32, "W2")
            b2b = load_const(b2b_d, (128, 16), F32, "b2b")
            W3s = load_const(W3_d, (16, 1), F32, "W3")
            b3c = load_const(b3c_d, (128, 1), F32, "b3c")
            wefs = [load_const(ins[f"wefs{li}"], (De, 2 * D), BF, f"wefs{li}")
                    for li in range(1, n_layers + 1)]
            wdst = [load_const(ins[f"wdst{li}"], (D, 2 * D), F32, f"wdst{li}")
                    for li in range(1, n_layers + 1)]
            wsrc = [load_const(ins[f"wsrc{li}"], (D, 2 * D), F32, f"wsrc{li}")
                    for li in range(1, n_layers + 1)]
            bdst = [load_const(ins[f"bdst{li}"], (128, 2 * D), F32, f"bdst{li}")
                    for li in range(1, n_layers + 1)]
            gw = [load_const(ins[f"gw{li}"], (128, 1), F32, f"gw{li}")
                  for li in range(1, n_layers + 1)]
            gb = [load_const(ins[f"gb{li}"], (128, 1), F32, f"gb{li}")
                  for li in range(1, n_layers + 1)]
            W1a = load_const(W1a_d, (Din1, D), F32, "W1a")

            hT = spool.tile([128, NPP], F32, tag="hT")

            for rep in range(repeat):
                # ======== FC1: hT = (x @ W1 + b1).T  (feature-major) ========
                for b in range(NBLK):
                    xTb = wpool.tile([Din1, BLK], F32, tag="xTb")
                    nc.sync.dma_start(xTb[:], xT_d[:, b * BLK : (b + 1) * BLK])
                    ps = psC.tile([128, BLK], F32, tag="agg", name="tp", space="PSUM")
                    nc.tensor.matmul(
                        ps[:], lhsT=W1a[:], rhs=xTb[:],
                        start=True, stop=True,
                    )
                    nc.vector.tensor_copy(out=hT[:, b * BLK : (b + 1) * BLK], in_=ps[:])

                for li in range(n_layers):
                    # ======== projections -> tables (streamed per block) ========
                    for b in range(NBLK):
                        hsl = hT[:, b * BLK : (b + 1) * BLK]
                        psS = psB.tile([128, 2 * D], F32, tag="pe", space="PSUM")
                        nc.tensor.matmul(psS[:], lhsT=hsl, rhs=wsrc[li][:],
                                         start=True, stop=True)
                        stS = wpool.tile([128, 2 * D], BF, tag="stS")
                        nc.vector.tensor_copy(out=stS[:], in_=psS[:])
                        nc.sync.dma_start(ag_in[li][b * BLK : (b + 1) * BLK, :], stS[:])
                        psDt = psB.tile([128, 2 * D], F32, tag="pe", space="PSUM")
                        nc.tensor.matmul(psDt[:], lhsT=hsl, rhs=wdst[li][:],
                                         start=True, stop=True)
                        stD = wpool.tile([128, 2 * D], BF, tag="stD")
                        nc.vector.tensor_tensor(out=stD[:], in0=psDt[:],
                                                in1=bdst[li][:], op=OP.add)
                        nc.sync.dma_start(dtab[li][b * BLK : (b + 1) * BLK, :], stD[:])
                    nc.gpsimd.collective_compute(
                        "AllGather", OP.bypass, replica_groups=rg,
                        ins=[ag_in[li][:]], outs=[table[li][:]],
                    )

                    # ======== edge phase ========
                    tabA = table[li][0:HALF, :]
                    tabB = table[li][HALF : 2 * HALF, :]

                    gA = [None] * max(NCA, 1)
                    gB = [None] * max(NCB, 1)
                    gD = [None] * NCD
                    ech = [None] * NCE

                    def get_gchunk(lst, k, tab_ap, idx_t, tag):
                        if lst[k] is None:
                            g = gpool.tile([128, GCHUNK // BLK, 2 * D], BF, tag=tag)
                            cols = GCHUNK // 16
                            nc.gpsimd.dma_gather(
                                g[:], tab_ap, idx_t[:, k * cols : (k + 1) * cols],
                                GCHUNK, GCHUNK, 2 * D,
                            )
                            lst[k] = g
                        return lst[k]

                    # tile t -> block id
                    t2b = []
                    for b in range(NBLK):
                        t2b += [b] * int(nA[b] + nB[b])
                    # tile t -> (is_A, stream pos)
                    t2g = []
                    pa = 0
                    pb = 0
                    for b in range(NBLK):
                        for _ in range(int(nA[b])):
                            t2g.append((True, pa))
                            pa += 1
                        for _ in range(int(nB[b])):
                            t2g.append((False, pb))
                            pb += 1
                    assert pa == TA and pb == TB and len(t2b) == TOT

                    BT = 32   # batch tiles (uv ring + batched DVE/ACT ops)
                    GRP = 4   # tiles per PSUM group (2 banks)
                    agg_of = {}

                    def finish_block(b):
                        aggT = agg_of.pop(b)
                        nc.vector.tensor_tensor(
                            out=hT[:, b * BLK : (b + 1) * BLK],
                            in0=hT[:, b * BLK : (b + 1) * BLK],
                            in1=aggT[:], op=OP.add,
                        )

                    for t0 in range(0, TOT, BT):
                        bt = min(BT, TOT - t0)
                        # uv ring: per tile [u | v] = [e^(-F) | e^(S)]
                        uv = rpool.tile([128, BT, 2 * D], BF, tag="uv")
                        ngrp = _ceil_div(bt, GRP)
                        for g0 in range(ngrp):
                            gw_ = min(GRP, bt - g0 * GRP)
                            psE = psB.tile([128, GRP * 2 * D], F32, tag="pe",
                                           space="PSUM")
                            for j in range(g0 * GRP, g0 * GRP + gw_):
                                t = t0 + j
                                isA, pos = t2g[t]
                                if isA:
                                    g = get_gchunk(gA, pos // 8, tabA, idxA, "gA")
                                else:
                                    g = get_gchunk(gB, pos // 8, tabB, idxB, "gB")
                                gsl = g[:, pos % 8, :]
                                gd = get_gchunk(gD, t // 8, dtab[li][:], idxD, "gD")
                                gdsl = gd[:, t % 8, :]
                                if ech[t // ECHUNK_TILES] is None:
                                    c0 = (t // ECHUNK_TILES) * ECHUNK_TILES * BLK
                                    cw = min(ECHUNK_TILES * BLK, SLOTS - c0)
                                    e = epool.tile([De, cw], BF, tag="ech")
                                    nc.sync.dma_start(e[:], eT_d[:, c0 : c0 + cw])
                                    ech[t // ECHUNK_TILES] = e
                                e = ech[t // ECHUNK_TILES]
                                ec = (t % ECHUNK_TILES) * BLK
                                sl = slice((j % GRP) * 2 * D, (j % GRP + 1) * 2 * D)
                                nc.tensor.matmul(
                                    psE[:, sl], lhsT=e[:, ec : ec + BLK],
                                    rhs=wefs[li][:], start=True, stop=False,
                                )
                                nc.tensor.matmul(psE[:, sl], lhsT=identb[:], rhs=gsl,
                                                 start=False, stop=False)
                                nc.tensor.matmul(psE[:, sl], lhsT=identb[:], rhs=gdsl,
                                                 start=False, stop=True)
                            # one exp over the whole group: uv = e^(psE)
                            nc.scalar.activation(
                                uv[:, g0 * GRP : g0 * GRP + gw_, :],
                                psE[:, : gw_ * 2 * D], AF.Exp,
                            )
                        # sp = ln(v + 1)  (in place, S-halves)
                        nc.scalar.activation(uv[:, :bt, D:], uv[:, :bt, D:],
                                             AF.Ln, bias=1.0)
                        with nc.allow_low_precision(reason="bf16 edge pipeline"):
                            # sigma = 1 / (1 + u)  (in place, F-halves)
                            nc.vector.tensor_scalar(
                                out=uv[:, :bt, :D], in0=uv[:, :bt, :D],
                                scalar1=1.0, scalar2=None, op0=OP.add,
                            )
                            nc.vector.reciprocal(uv[:, :bt, :D], uv[:, :bt, :D])
                            # m = sigma * sp  (in place into F-halves)
                            nc.vector.tensor_tensor(out=uv[:, :bt, :D],
                                                    in0=uv[:, :bt, :D],
                                                    in1=uv[:, :bt, D:], op=OP.mult)
                        ohr = rpool.tile([128, BT, 128], BF, tag="ohr")
                        nc.vector.tensor_tensor(
                            out=ohr[:, :bt, :],
                            in0=iota[:].rearrange("p (o f) -> p o f", o=1)
                                .to_broadcast([128, bt, 128]),
                            in1=dloc[:, t0 : t0 + bt]
                                .rearrange("p (t o) -> p t o", o=1)
                                .to_broadcast([128, bt, 128]),
                            op=OP.is_equal,
                        )
                        for j in range(bt):
                            t = t0 + j
                            b = t2b[t]
                            if b not in agg_of:
                                agg_of[b] = psC.tile([128, BLK], F32, tag="agg",
                                                     name=f"agg{b}", space="PSUM")
                            last = (t == TOT - 1) or (t2b[t + 1] != b)
                            nc.tensor.matmul(
                                agg_of[b][:], lhsT=uv[:, j, :D],
                                rhs=ohr[:, j, :],
                                start=(t == 0) or (t2b[t - 1] != b), stop=last,
                            )
                            if last:
                                finish_block(b)


                    # ======== LayerNorm (graph mode: global stats) ========
                    NPr = sched["NP"]
                    stats = spool.tile([128, 2], F32, tag="stats")
                    nc.vector.reduce_sum(stats[:, 0:1], hT[:, :NPr],
                                         axis=mybir.AxisListType.X)
                    sq = wpool.tile([128, 512], F32, tag="sq")
                    sqacc = spool.tile([128, _ceil_div(NPr, 512)], F32, tag="sqacc")
                    for k in range(_ceil_div(NPr, 512)):
                        k0 = k * 512
                        k1 = min(NPr, k0 + 512)
                        nc.scalar.activation(sq[:, : k1 - k0], hT[:, k0:k1], AF.Square)
                        nc.vector.reduce_sum(sqacc[:, k : k + 1], sq[:, : k1 - k0],
                                             axis=mybir.AxisListType.X)
                    nc.vector.reduce_sum(stats[:, 1:2], sqacc[:],
                                         axis=mybir.AxisListType.X)
                    psR = psC.tile([128, BLK], F32, tag="agg", name="tp", space="PSUM")
                    nc.tensor.matmul(psR[:1, :2], lhsT=ones_col[:], rhs=stats[:],
                                     start=True, stop=True)
                    stl = wpool.tile([1, 2], F32, tag="stl")
                    nc.vector.tensor_copy(out=stl[:], in_=psR[:1, :2])
                    nc.sync.dma_start(st_in[li][:], stl[:])
                    nc.gpsimd.collective_compute(
                        "AllReduce", OP.add, replica_groups=rg,
                        ins=[st_in[li][:]], outs=[st_out[li][:]],
                    )
                    stg = wpool.tile([1, 2], F32, tag="stl")
                    nc.sync.dma_start(stg[:], st_out[li][:])
                    psBc = psC.tile([128, BLK], F32, tag="agg", name="tp", space="PSUM")
                    nc.tensor.matmul(psBc[:, :2], lhsT=ones_row[:], rhs=stg[:],
                                     start=True, stop=True)
                    stb = wpool.tile([128, 2], F32, tag="stb")
                    # mean/E[x^2]
                    nc.vector.tensor_scalar(out=stb[:], in0=psBc[:, :2],
                                            scalar1=inv_ND, scalar2=None, op0=OP.mult)
                    mean = stb[:, 0:1]
                    ex2 = stb[:, 1:2]
                    var = wpool.tile([128, 1], F32, tag="v1")
                    nc.vector.tensor_tensor(out=var[:], in0=mean, in1=mean, op=OP.mult)
                    nc.vector.tensor_tensor(out=var[:], in0=ex2, in1=var[:],
                                            op=OP.subtract)
                    std = wpool.tile([128, 1], F32, tag="v2")
                    nc.scalar.activation(std[:], var[:], AF.Sqrt)
                    nc.vector.tensor_scalar(out=std[:], in0=std[:], scalar1=float(EPS),
                                            scalar2=None, op0=OP.add)
                    inv = wpool.tile([128, 1], F32, tag="v3")
                    nc.vector.reciprocal(inv[:], std[:])
                    scale_c = wpool.tile([128, 1], F32, tag="v4")
                    nc.vector.tensor_tensor(out=scale_c[:], in0=inv[:], in1=gw[li][:],
                                            op=OP.mult)
                    bias_c = wpool.tile([128, 1], F32, tag="v5")
                    nc.vector.tensor_tensor(out=bias_c[:], in0=mean, in1=scale_c[:],
                                            op=OP.mult)
                    nc.vector.tensor_tensor(out=bias_c[:], in0=gb[li][:], in1=bias_c[:],
                                            op=OP.subtract)
                    # apply + relu
                    for k in range(_ceil_div(NPP, 512)):
                        k0 = k * 512
                        k1 = min(NPP, k0 + 512)
                        tmp = wpool.tile([128, 512], F32, tag="lnt")
                        nc.vector.tensor_scalar(
                            out=tmp[:, : k1 - k0], in0=hT[:, k0:k1],
                            scalar1=scale_c[:], scalar2=bias_c[:],
                            op0=OP.mult, op1=OP.add,
                        )
                        nc.scalar.activation(hT[:, k0:k1], tmp[:, : k1 - k0], AF.Relu)

                # ======== pooling + MLP ========
                pool_ps = psC.tile([128, D], F32, tag="agg", space="PSUM")
                for b in range(NBLK):
                    psT = psC.tile([128, BLK], F32, tag="agg", name="tp", space="PSUM")
                    nc.tensor.transpose(psT[:], hT[:, b * BLK : (b + 1) * BLK], ident[:])
                    h3 = wpool.tile([128, D], BF, tag="h3")
                    nc.vector.tensor_copy(out=h3[:], in_=psT[:])
                    ohg = wpool.tile([128, 128], BF, tag="oh")
                    nc.vector.tensor_scalar(
                        out=ohg[:], in0=iota[:], scalar1=bcols[:, b : b + 1],
                        scalar2=None, op0=OP.is_equal,
                    )
                    nc.tensor.matmul(pool_ps[:], lhsT=ohg[:], rhs=h3[:],
                                     start=(b == 0), stop=(b == NBLK - 1))
                pool_sb = wpool.tile([G, D], F32, tag="poolsb")
                nc.vector.tensor_copy(out=pool_sb[:], in_=pool_ps[:])
                nc.sync.dma_start(pool_in[:], pool_sb[:])
                nc.gpsimd.collective_compute(
                    "AllReduce", OP.add, replica_groups=rg,
                    ins=[pool_in[:]], outs=[pool_out[:]],
                )
                hg = wpool.tile([G, D], F32, tag="poolsb")
                nc.sync.dma_start(hg[:], pool_out[:])
                nc.vector.tensor_scalar(out=hg[:], in0=hg[:], scalar1=invc[:],
                                        scalar2=None, op0=OP.mult)
                psT2 = psC.tile([128, BLK], F32, tag="agg", name="tp", space="PSUM")
                nc.tensor.transpose(psT2[:], hg[:], ident[:])
                hgT = wpool.tile([D, G], F32, tag="hgT")
                nc.vector.tensor_copy(out=hgT[:], in_=psT2[:])
                ps2 = psC.tile([128, BLK], F32, tag="agg", name="tp", space="PSUM")
                nc.tensor.matmul(ps2[:, :16], lhsT=hgT[:], rhs=W2s[:],
                                 start=True, stop=True)
                h2 = wpool.tile([G, 16], F32, tag="h2")
                nc.vector.tensor_tensor(out=h2[:], in0=ps2[:, :16], in1=b2b[:],
                                        op=OP.add)
                nc.scalar.activation(h2[:], h2[:], AF.Relu)
                psT3 = psC.tile([128, BLK], F32, tag="agg", name="tp", space="PSUM")
                nc.tensor.transpose(psT3[:16, :], h2[:], ident[:])
                h2T = wpool.tile([16, G], F32, tag="h2T")
                nc.vector.tensor_copy(out=h2T[:], in_=psT3[:16, :])
                ps3 = psC.tile([128, BLK], F32, tag="agg", name="tp", space="PSUM")
                nc.tensor.matmul(ps3[:, :1], lhsT=h2T[:], rhs=W3s[:],
                                 start=True, stop=True)
                outsb = wpool.tile([G, 1], F32, tag="outsb")
                nc.vector.tensor_tensor(out=outsb[:], in0=ps3[:, :1], in1=b3c[:],
                                        op=OP.add)
                nc.sync.dma_start(out_d[:], outsb[:])

    _t1 = _time.time()
    print(f"[build_nc] trace: {_t1 - _t0:.1f}s, instrs: "
          f"{sum(len(bb.instructions) for bb in nc.main_func.blocks)}", flush=True)
    nc.compile()
    print(f"[build_nc] bass compile: {_time.time() - _t1:.1f}s", flush=True)
    return nc


def kernel(x, edge_index, edge_attr, batch,
           W1, b1,
           Wf1, bf1, Ws1, bs1, g1w, g1b,
           Wf2, bf2, Ws2, bs2, g2w, g2b,
           Wf3, bf3, Ws3, bs3, g3w, g3b,
           W2, b2, W3, b3):
    layer_params = [
        (Wf1, bf1, Ws1, bs1, g1w, g1b),
        (Wf2, bf2, Ws2, bs2, g2w, g2b),
        (Wf3, bf3, Ws3, bs3, g3w, g3b),
    ]
    sched, in_maps = host_prep(np.asarray(x), np.asarray(edge_index),
                               np.asarray(edge_attr), np.asarray(batch),
                               np.asarray(W1), np.asarray(b1), layer_params,
                               np.asarray(W2), np.asarray(b2),
                               np.asarray(W3), np.asarray(b3))
    import time as _time
    _tp = _time.time()
    nc = build_nc(sched)
    print(f"[kernel] build done {_time.time() - _tp:.1f}s", flush=True)
    _tr = _time.time()
    res = run_bass_kernel_spmd(nc, in_maps, core_ids=list(range(NC_CORES)))
    print(f"[kernel] run (incl neff compile) {_time.time() - _tr:.1f}s", flush=True)
    global LAST_RESULTS
    LAST_RESULTS = res
    return np.asarray(res.results[0]["out"], np.float32)


LAST_RESULTS = None

